# revision 1
# baseline (speedup 1.0000x reference)
"""Distributed Bass kernel for nn_Interaction_GraphConvolution.

Math (reference):
    x  = node_features @ linear_w.T + linear_b          [N, IN_F]
    wf = x @ weight                                     [N, C]
    G  = mask_father[:,0,:].T @ adjacency               [N, N]
    P  = G * mask_hadamard[:,0,:].T                     [N, N]
    out[c, j] = wf[j,c] * (P @ wf)[j,c] / neighbor_count[c]^2

Sharding: output columns j (node dim) split across 8 cores, 512 each.
Two SPMD launches:
  NEFF-1: core m computes wf rows J_m (512 rows). Host gathers full wf.
  NEFF-2: core m computes G^T/P^T columns J_m and out[:, J_m].
Dtypes: adjacency-side matmuls in bf16 (inputs are small ints - exact);
wf-side matmuls in float32r (~1.5e-4 rel err at full PE rate).
"""

import os
import sys

sys.path.insert(0, "/opt/trn_rl_repo")

import numpy as np
import ml_dtypes

from concourse import bass, bacc, mybir, tile
from concourse.bass_utils import run_bass_kernel_spmd
from concourse.masks import make_identity

F32 = mybir.dt.float32
F32R = mybir.dt.float32r
BF16 = mybir.dt.bfloat16

N = 4096       # nodes (== out channels C)
F_RAW = 512    # raw feature dim
IN_F = 1024    # hidden dim
C = 4096       # out channels
M = 8          # cores
JB = N // M    # 512 output columns per core

LAST_EXEC = {}
LAST_RESULTS = {}


def _build_neff1():
    """Per core: wf_rows[J_m] = (nf[J_m] @ lw.T + b) @ W, via transposed tiles.

    Inputs (per core): lwT [F_RAW, IN_F] f32r, nfT [F_RAW, JB] f32r,
    bias [128, IN_F//128] f32, w [IN_F, C] f32r.
    Output: wf_rows [JB, C] f32.
    """
    nc = bacc.Bacc()
    lwT_d = nc.dram_tensor("lwT", [F_RAW, IN_F], F32R, kind="ExternalInput")
    nfT_d = nc.dram_tensor("nfT", [F_RAW, JB], F32R, kind="ExternalInput")
    b_d = nc.dram_tensor("bias", [128, IN_F // 128], F32, kind="ExternalInput")
    w_d = nc.dram_tensor("w", [IN_F, C], F32R, kind="ExternalInput")
    wf_d = nc.dram_tensor("wf_rows", [JB, C], F32, kind="ExternalOutput")

    NFB = IN_F // 128   # 8 f-blocks
    NRB = F_RAW // 128  # 4 r-blocks
    NJB = JB // 128     # 4 j-blocks
    NCC = C // 512      # 8 c-chunks

    with tile.TileContext(nc) as tc:
        with tc.tile_pool(name="const", bufs=1) as constp:
            lwT_t = constp.tile([128, NRB * IN_F], F32R)
            for rb in range(NRB):
                nc.sync.dma_start(
                    lwT_t[:, rb * IN_F:(rb + 1) * IN_F],
                    lwT_d[rb * 128:(rb + 1) * 128, :])
            nfT_t = constp.tile([128, NRB * JB], F32R)
            for rb in range(NRB):
                nc.sync.dma_start(
                    nfT_t[:, rb * JB:(rb + 1) * JB],
                    nfT_d[rb * 128:(rb + 1) * 128, :])
            b_t = constp.tile([128, NFB], F32)
            nc.sync.dma_start(b_t[:], b_d[:])
            w_t = constp.tile([128, NFB * C], F32R)
            for fb in range(NFB):
                nc.sync.dma_start(
                    w_t[:, fb * C:(fb + 1) * C],
                    w_d[fb * 128:(fb + 1) * 128, :])
            xt_t = constp.tile([128, NFB * JB], F32R)

            # phase X: xT[f, j] = lw @ nf[J_m].T + b
            with tc.tile_pool(name="psx", bufs=2, space=bass.MemorySpace.PSUM) as psxp:
                for fb in range(NFB):
                    psx = psxp.tile([128, JB], F32, tag="psx")
                    for rb in range(NRB):
                        nc.tensor.matmul(
                            psx[:],
                            lwT_t[:, rb * IN_F + fb * 128: rb * IN_F + (fb + 1) * 128],
                            nfT_t[:, rb * JB:(rb + 1) * JB],
                            start=(rb == 0), stop=(rb == NRB - 1))
                    nc.scalar.activation(
                        xt_t[:, fb * JB:(fb + 1) * JB], psx[:],
                        mybir.ActivationFunctionType.Identity,
                        bias=b_t[:, fb:fb + 1], scale=1.0)

            # phase W: wf[J_m] = xT.T @ W
            with tc.tile_pool(name="psw", bufs=8, space=bass.MemorySpace.PSUM) as pswp, \
                 tc.tile_pool(name="io1", bufs=3) as iop:
                for jb in range(NJB):
                    for cc in range(NCC):
                        pw = pswp.tile([128, 512], F32, tag="pw")
                        for fb in range(NFB):
                            nc.tensor.matmul(
                                pw[:],
                                xt_t[:, fb * JB + jb * 128: fb * JB + (jb + 1) * 128],
                                w_t[:, fb * C + cc * 512: fb * C + (cc + 1) * 512],
                                start=(fb == 0), stop=(fb == NFB - 1))
                        o_sb = iop.tile([128, 512], F32, tag="o_sb")
                        nc.vector.tensor_copy(o_sb[:], pw[:])
                        nc.sync.dma_start(
                            wf_d[jb * 128:(jb + 1) * 128, cc * 512:(cc + 1) * 512],
                            o_sb[:])
    nc.finalize()
    return nc


def _build_neff2():
    """Per core: G^T/P^T for columns J_m, then out[:, J_m].

    Inputs: a [N, N] bf16 (adjacency), ao [N, JB] bf16 (mask_father cols),
    s [N, JB] bf16 (mask_hadamard cols), wfd [N, C] f32r (full wf),
    wfs [JB, C] f32 (wf rows J_m, pre-scaled by nothing - raw),
    inv2 [128, N//128] f32 (1/neighbor_count^2 tiled).
    Output: outc [C, JB] f32  (= output[:, J_m]).
    """
    nc = bacc.Bacc()
    a_d = nc.dram_tensor("a", [N, N], BF16, kind="ExternalInput")
    ao_d = nc.dram_tensor("ao", [N, JB], BF16, kind="ExternalInput")
    s_d = nc.dram_tensor("s", [N, JB], BF16, kind="ExternalInput")
    wf_d = nc.dram_tensor("wfd", [N, C], F32R, kind="ExternalInput")
    wr_d = nc.dram_tensor("wfs", [JB, C], F32, kind="ExternalInput")
    i2_d = nc.dram_tensor("inv2", [128, N // 128], F32, kind="ExternalInput")
    out_d = nc.dram_tensor("outc", [C, JB], F32, kind="ExternalOutput")

    NKB = N // 128    # 32 k-blocks
    NIB = N // 128    # 32 i-blocks
    NCB = C // 128    # 32 c-blocks
    NJB = JB // 128   # 4 j-blocks

    with tile.TileContext(nc) as tc:
        with tc.tile_pool(name="const", bufs=1) as constp:
            ident = constp.tile([128, 128], F32)
            make_identity(nc, ident[:])
            i2_t = constp.tile([128, N // 128], F32)
            nc.sync.dma_start(i2_t[:], i2_d[:])
            aot = constp.tile([128, NKB * JB], BF16)
            for kb in range(NKB):
                nc.sync.dma_start(
                    aot[:, kb * JB:(kb + 1) * JB],
                    ao_d[kb * 128:(kb + 1) * 128, :])
            pt_t = constp.tile([128, NIB * JB], F32R)

            # phase G: PT[i, j] = (A^T @ Ao) * S  for j in J_m
            with tc.tile_pool(name="psg", bufs=8, space=bass.MemorySpace.PSUM) as psgp, \
                 tc.tile_pool(name="ioa", bufs=3) as ioa, \
                 tc.tile_pool(name="ios", bufs=2) as ios:
                for isup in range(NIB // 8):
                    psg = [psgp.tile([128, JB], F32, tag="psg", name=f"psg{_i}") for _i in range(8)]
                    for kb in range(NKB):
                        a_t = ioa.tile([128, 1024], BF16, tag="a_t")
                        nc.sync.dma_start(
                            a_t[:],
                            a_d[kb * 128:(kb + 1) * 128,
                                isup * 1024:(isup + 1) * 1024])
                        for ib8 in range(8):
                            nc.tensor.matmul(
                                psg[ib8][:],
                                a_t[:, ib8 * 128:(ib8 + 1) * 128],
                                aot[:, kb * JB:(kb + 1) * JB],
                                start=(kb == 0), stop=(kb == NKB - 1))
                    for ib8 in range(8):
                        ib = isup * 8 + ib8
                        s_t = ios.tile([128, JB], BF16, tag="s_t")
                        nc.sync.dma_start(s_t[:], s_d[ib * 128:(ib + 1) * 128, :])
                        nc.vector.tensor_mul(
                            pt_t[:, ib * JB:(ib + 1) * JB], psg[ib8][:], s_t[:])

            # phase O: out[c, j] = (wf^T @ PT) * wf^T * inv2
            with tc.tile_pool(name="pso", bufs=4, space=bass.MemorySpace.PSUM) as psop, \
                 tc.tile_pool(name="pst", bufs=2, space=bass.MemorySpace.PSUM) as pstp, \
                 tc.tile_pool(name="iow", bufs=4) as iow, \
                 tc.tile_pool(name="ior", bufs=4) as ior, \
                 tc.tile_pool(name="ioo", bufs=3) as ioo:
                for csup in range(NCB // 4):
                    pso = [psop.tile([128, JB], F32, tag="pso", name=f"pso{_i}") for _i in range(4)]
                    for ib in range(NIB):
                        wf_t = iow.tile([128, 512], F32R, tag="wf_t")
                        nc.sync.dma_start(
                            wf_t[:],
                            wf_d[ib * 128:(ib + 1) * 128,
                                 csup * 512:(csup + 1) * 512])
                        for cb4 in range(4):
                            nc.tensor.matmul(
                                pso[cb4][:],
                                wf_t[:, cb4 * 128:(cb4 + 1) * 128],
                                pt_t[:, ib * JB:(ib + 1) * JB],
                                start=(ib == 0), stop=(ib == NIB - 1))
                    for cb4 in range(4):
                        cb = csup * 4 + cb4
                        ptp = pstp.tile([128, JB], F32, tag="ptp")
                        for jb in range(NJB):
                            wr_t = ior.tile([128, 128], F32, tag="wr_t")
                            nc.sync.dma_start(
                                wr_t[:],
                                wr_d[jb * 128:(jb + 1) * 128,
                                     cb * 128:(cb + 1) * 128])
                            nc.tensor.transpose(
                                ptp[:, jb * 128:(jb + 1) * 128], wr_t[:], ident[:])
                        wt_sb = ioo.tile([128, JB], F32, tag="wt_sb")
                        nc.scalar.activation(
                            wt_sb[:], ptp[:],
                            mybir.ActivationFunctionType.Identity,
                            bias=0.0, scale=i2_t[:, cb:cb + 1])
                        o_sb = ioo.tile([128, JB], F32, tag="o_sb")
                        nc.vector.tensor_mul(o_sb[:], pso[cb4][:], wt_sb[:])
                        nc.sync.dma_start(out_d[cb * 128:(cb + 1) * 128, :], o_sb[:])
    nc.finalize()
    return nc


_NC1 = None
_NC2 = None


def _get_ncs():
    global _NC1, _NC2
    if _NC1 is None:
        _NC1 = _build_neff1()
        _NC2 = _build_neff2()
    return _NC1, _NC2


def _ensure_trace_hook():
    """Best-effort NTFF profiling shim (test harness only; grading runs
    without tracing). The agent image's antenv lacks axon_hooks, but the
    axon boot package exposes the ctypes equivalent."""
    try:
        from antenv.axon_hooks import get_axon_ntff_profile_hook
        return get_axon_ntff_profile_hook() is not None
    except ImportError:
        pass
    try:
        import types
        if "/root/.axon_site" not in sys.path:
            sys.path.insert(0, "/root/.axon_site")
        from trn_agent_boot.trn_boot import _ntff_profile_via_ctypes
        hook = _ntff_profile_via_ctypes("/opt/axon/libaxon_pjrt.so")
        if hook is None:
            return False
        import antenv
        mod = types.ModuleType("antenv.axon_hooks")
        mod.get_axon_ntff_profile_hook = lambda: hook
        mod.set_axon_ntff_profile_hook = lambda h: None
        sys.modules["antenv.axon_hooks"] = mod
        antenv.axon_hooks = mod
        from concourse import bass_utils as _bu
        _bu.upload_artifacts = lambda tmpdir: ""
        return True
    except Exception:
        return False


def _run(nc, in_maps, cores, trace, tag):
    if trace:
        try:
            r = run_bass_kernel_spmd(nc, in_maps, cores, trace=True)
            LAST_EXEC[tag] = r.exec_time_ns
            LAST_RESULTS[tag] = r
            return r
        except Exception as e:
            print(f"trace run failed ({e!r}); retrying without trace")
    return run_bass_kernel_spmd(nc, in_maps, cores)


def kernel(node_features, adjacency_matrix, mask_father, neighbor_count,
           mask_hadamard, linear_w, linear_b, weight):
    nc1, nc2 = _get_ncs()
    trace = bool(int(os.environ.get("BASS_KERNEL_TRACE", "0"))) and _ensure_trace_hook()
    cores = list(range(M))
    bf = ml_dtypes.bfloat16

    nf = np.ascontiguousarray(np.asarray(node_features, dtype=np.float32))
    A = np.ascontiguousarray(np.asarray(adjacency_matrix, dtype=np.float32))
    Ao = np.ascontiguousarray(np.asarray(mask_father, dtype=np.float32)[:, 0, :])
    S = np.ascontiguousarray(np.asarray(mask_hadamard, dtype=np.float32)[:, 0, :])
    ncnt = np.asarray(neighbor_count, dtype=np.float32)
    lw = np.asarray(linear_w, dtype=np.float32)
    lb = np.asarray(linear_b, dtype=np.float32)
    W = np.ascontiguousarray(np.asarray(weight, dtype=np.float32))

    # ---- launch 1: wf rows ----
    lwT = np.ascontiguousarray(lw.T)                       # [F_RAW, IN_F]
    bias = np.ascontiguousarray(lb.reshape(IN_F // 128, 128).T)  # [128, 8]
    in1 = []
    for m in range(M):
        nfT = np.ascontiguousarray(nf[m * JB:(m + 1) * JB, :].T)  # [F_RAW, JB]
        in1.append({"lwT": lwT, "nfT": nfT, "bias": bias, "w": W})
    r1 = _run(nc1, in1, cores, trace, "neff1")
    wf = np.concatenate([r1.results[m]["wf_rows"] for m in range(M)], axis=0)

    # ---- launch 2: graph conv ----
    A_b = A.astype(bf)
    inv2 = (1.0 / np.square(ncnt.astype(np.float64)))[:, 0].astype(np.float32)
    inv2_t = np.ascontiguousarray(inv2.reshape(N // 128, 128).T)  # [128, 32]
    in2 = []
    for m in range(M):
        sl = slice(m * JB, (m + 1) * JB)
        in2.append({
            "a": A_b,
            "ao": np.ascontiguousarray(Ao[:, sl]).astype(bf),
            "s": np.ascontiguousarray(S[:, sl]).astype(bf),
            "wfd": wf,
            "wfs": np.ascontiguousarray(wf[sl, :]),
            "inv2": inv2_t,
        })
    r2 = _run(nc2, in2, cores, trace, "neff2")

    out = np.empty((C, N), dtype=np.float32)
    for m in range(M):
        out[:, m * JB:(m + 1) * JB] = r2.results[m]["outc"]
    return out



# revision 3
# speedup vs baseline: 1.6358x; 1.6358x over previous
"""Distributed Bass kernel for nn_Interaction_GraphConvolution.

Math (reference):
    x  = node_features @ linear_w.T + linear_b          [N, IN_F]
    wf = x @ weight                                     [N, C]
    G  = mask_father[:,0,:].T @ adjacency               [N, N]
    P  = G * mask_hadamard[:,0,:].T                     [N, N]
    out[c, j] = wf[j,c] * (P @ wf)[j,c] / neighbor_count[c]^2

Host folds the two linear layers: FW = lw.T @ W, fb = lb @ W, so
wf = nf @ FW + fb (one on-device GEMM).  Output columns j (node dim)
are split across 8 cores, 512 each.  Two SPMD launches:
  NEFF-1: core m computes wf rows J_m (512 rows) in bf16.
  NEFF-2: core m computes G rows J_m (fp8 DoubleRow - adjacency is 0/1,
          exact), multiplies by S^T, transposes to P^T on device, then
          out[j in J_m, c] = (P@wf)[j,c] * (wf[j,c]/ncnt[c]^2).
Matmuls keep one 128x128 stationary tile across many 512-wide moving
chunks so LDWEIGHTS amortizes instead of serializing per matmul.
"""

import os
import sys

sys.path.insert(0, "/opt/trn_rl_repo")

import numpy as np
import ml_dtypes

from concourse import bass, bacc, mybir, tile
from concourse.bass_utils import run_bass_kernel_spmd
from concourse.masks import make_identity

F32 = mybir.dt.float32
BF16 = mybir.dt.bfloat16
F8E4 = mybir.dt.float8e4
DR = mybir.MatmulPerfMode.DoubleRow

N = 4096       # nodes (== out channels C)
F_RAW = 512    # raw feature dim
IN_F = 1024    # hidden dim
C = 4096       # out channels
M = 8          # cores
JB = N // M    # 512 output columns per core

LAST_EXEC = {}
LAST_RESULTS = {}


def _build_neff1():
    """Per core: wf[J_m, :] = nf[J_m] @ FW + fb, output bf16.

    Inputs: nfT [F_RAW, JB] bf16 (nf rows J_m, transposed),
    fw [F_RAW, C] bf16 (host-fused lw.T @ W), fbt [128, C] f32
    (fb broadcast to all partitions).  Output: wfb [JB, C] bf16.
    """
    nc = bacc.Bacc()
    nfT_d = nc.dram_tensor("nfT", [F_RAW, JB], BF16, kind="ExternalInput")
    fw_d = nc.dram_tensor("fw", [F_RAW, C], BF16, kind="ExternalInput")
    fbt_d = nc.dram_tensor("fbt", [128, C], F32, kind="ExternalInput")
    wf_d = nc.dram_tensor("wfb", [JB, C], BF16, kind="ExternalOutput")

    NRB = F_RAW // 128  # 4 contraction blocks
    NJB = JB // 128     # 4 j blocks
    CH = 2048           # c half

    with tile.TileContext(nc) as tc:
        with tc.tile_pool(name="const", bufs=1) as constp, \
             tc.tile_pool(name="fwp", bufs=2) as fwp, \
             tc.tile_pool(name="ps", bufs=8, space=bass.MemorySpace.PSUM) as psp, \
             tc.tile_pool(name="io", bufs=3) as iop:
            nfT_t = constp.tile([128, NRB, JB], BF16)
            for rb in range(NRB):
                nc.sync.dma_start(nfT_t[:, rb, :], nfT_d[rb * 128:(rb + 1) * 128, :])
            fbt_t = constp.tile([128, C], F32)
            nc.sync.dma_start(fbt_t[:], fbt_d[:])
            for ch in range(C // CH):
                fw_t = fwp.tile([128, NRB, CH], BF16, tag="fw", name=f"fw{ch}")
                for rb in range(NRB):
                    nc.sync.dma_start(
                        fw_t[:, rb, :],
                        fw_d[rb * 128:(rb + 1) * 128, ch * CH:(ch + 1) * CH])
                for jb in range(NJB):
                    ps = [psp.tile([128, 512], F32, tag="ps", name=f"ps{ch}_{jb}_{i}")
                          for i in range(4)]
                    for rb in range(NRB):
                        lhs = nfT_t[:, rb, jb * 128:(jb + 1) * 128]
                        for cc in range(4):
                            nc.tensor.matmul(
                                ps[cc][:], lhs,
                                fw_t[:, rb, cc * 512:(cc + 1) * 512],
                                start=(rb == 0), stop=(rb == NRB - 1))
                    o_sb = iop.tile([128, CH], BF16, tag="o_sb", name=f"o{ch}_{jb}")
                    for cc in range(4):
                        nc.vector.tensor_add(
                            o_sb[:, cc * 512:(cc + 1) * 512], ps[cc][:],
                            fbt_t[:, ch * CH + cc * 512: ch * CH + (cc + 1) * 512])
                    nc.sync.dma_start(
                        wf_d[jb * 128:(jb + 1) * 128, ch * CH:(ch + 1) * CH],
                        o_sb[:])
    nc.finalize()
    return nc


def _build_neff2():
    """Per core: out[j in J_m, c] = (P @ wf)[j, c] * wfs[j, c].

    Inputs: a8 [N, N] fp8e4 (adjacency), ao8 [N, JB] fp8e4 (mask_father
    cols J_m), stT [JB, N] bf16 (mask_hadamard cols J_m, transposed),
    wfd [N, C] bf16 (full wf), wfs [JB, C] f32 (wf rows * 1/ncnt^2).
    Output: outc [JB, C] f32 (transposed vs final; host untransposes).
    """
    nc = bacc.Bacc()
    a_d = nc.dram_tensor("a8", [N, N], F8E4, kind="ExternalInput")
    ao_d = nc.dram_tensor("ao8", [N, JB], F8E4, kind="ExternalInput")
    st_d = nc.dram_tensor("stT", [JB, N], BF16, kind="ExternalInput")
    wfd_d = nc.dram_tensor("wfd", [N, C], BF16, kind="ExternalInput")
    wfs_d = nc.dram_tensor("wfs", [JB, C], F32, kind="ExternalInput")
    out_d = nc.dram_tensor("outc", [JB, C], F32, kind="ExternalOutput")

    NKP = N // 256    # 16 k-pairs (DoubleRow contracts 256 per pass)
    NIB = N // 128    # 32 i-blocks
    NJB = JB // 128   # 4 j-blocks
    NIC = N // 512    # 8 i-chunks of 512
    CQ = 1024         # c quarter
    NCQ = C // CQ     # 4

    with tile.TileContext(nc) as tc:
        with tc.tile_pool(name="const", bufs=1) as constp:
            ident = constp.tile([128, 128], BF16)
            make_identity(nc, ident[:])
            # P^T, [i, j] with i on partitions: ptT[:, ib, j] = P[j, ib*128+p]
            ptT_t = constp.tile([128, NIB, JB], BF16)

            # ---- phase G: G rows J_m (fp8 DoubleRow), *S^T, transpose ----
            with tc.tile_pool(name="ga", bufs=1) as gap, \
                 tc.tile_pool(name="stp", bufs=1) as stp, \
                 tc.tile_pool(name="pgp", bufs=1) as pgp:
                aot_t = gap.tile([128, NKP, 2, JB], F8E4)
                for kb in range(2 * NKP):
                    nc.sync.dma_start(
                        aot_t[:, kb // 2, kb % 2, :],
                        ao_d[kb * 128:(kb + 1) * 128, :])
                a_t = gap.tile([128, NKP, 2, N], F8E4)
                for kb in range(2 * NKP):
                    nc.sync.dma_start(
                        a_t[:, kb // 2, kb % 2, :],
                        a_d[kb * 128:(kb + 1) * 128, :])

                for jb in range(NJB):
                    st_t = stp.tile([128, N], BF16, tag="st", name=f"st{jb}")
                    nc.sync.dma_start(st_t[:], st_d[jb * 128:(jb + 1) * 128, :])
                    pg_sb = pgp.tile([128, N], BF16, tag="pg", name=f"pg{jb}")
                    with tc.tile_pool(name=f"psG{jb}", bufs=8,
                                      space=bass.MemorySpace.PSUM) as psgp:
                        psg = [psgp.tile([128, 512], F32, tag="g",
                                         name=f"g{jb}_{i}") for i in range(NIC)]
                        for kp in range(NKP):
                            lhs = aot_t[:, kp, :, jb * 128:(jb + 1) * 128]
                            for ic in range(NIC):
                                nc.tensor.matmul(
                                    psg[ic][:], lhs,
                                    a_t[:, kp, :, ic * 512:(ic + 1) * 512],
                                    start=(kp == 0), stop=(kp == NKP - 1),
                                    perf_mode=DR)
                        for ic in range(NIC):
                            nc.vector.tensor_mul(
                                pg_sb[:, ic * 512:(ic + 1) * 512], psg[ic][:],
                                st_t[:, ic * 512:(ic + 1) * 512])
                    with tc.tile_pool(name=f"psT{jb}", bufs=4,
                                      space=bass.MemorySpace.PSUM) as pstp:
                        for ib in range(NIB):
                            pst = pstp.tile([128, 128], BF16, tag="t",
                                            name=f"t{jb}_{ib}")
                            nc.tensor.transpose(
                                pst[:], pg_sb[:, ib * 128:(ib + 1) * 128],
                                ident[:])
                            nc.scalar.copy(
                                ptT_t[:, ib, jb * 128:(jb + 1) * 128], pst[:])

            # ---- phase O: out[j, c] = (P @ wf) * wfs, c-quarter resident ----
            with tc.tile_pool(name="wfp", bufs=2) as wfp, \
                 tc.tile_pool(name="wfsp", bufs=2) as wfsp, \
                 tc.tile_pool(name="osp", bufs=2) as osp, \
                 tc.tile_pool(name="psO", bufs=8,
                              space=bass.MemorySpace.PSUM) as psop:
                for ch in range(NCQ):
                    wf_t = wfp.tile([128, NIB, CQ], BF16, tag="wf", name=f"wf{ch}")
                    for ib in range(NIB):
                        nc.sync.dma_start(
                            wf_t[:, ib, :],
                            wfd_d[ib * 128:(ib + 1) * 128, ch * CQ:(ch + 1) * CQ])
                    for jb in range(NJB):
                        wfs_t = wfsp.tile([128, CQ], F32, tag="wfs",
                                          name=f"wfs{ch}_{jb}")
                        nc.sync.dma_start(
                            wfs_t[:],
                            wfs_d[jb * 128:(jb + 1) * 128, ch * CQ:(ch + 1) * CQ])
                        ps = [psop.tile([128, 512], F32, tag="po",
                                        name=f"po{ch}_{jb}_{i}") for i in range(2)]
                        for ib in range(NIB):
                            lhs = ptT_t[:, ib, jb * 128:(jb + 1) * 128]
                            for cc in range(2):
                                nc.tensor.matmul(
                                    ps[cc][:], lhs,
                                    wf_t[:, ib, cc * 512:(cc + 1) * 512],
                                    start=(ib == 0), stop=(ib == NIB - 1))
                        o_sb = osp.tile([128, CQ], F32, tag="o", name=f"o{ch}_{jb}")
                        for cc in range(2):
                            nc.vector.tensor_mul(
                                o_sb[:, cc * 512:(cc + 1) * 512], ps[cc][:],
                                wfs_t[:, cc * 512:(cc + 1) * 512])
                        nc.sync.dma_start(
                            out_d[jb * 128:(jb + 1) * 128, ch * CQ:(ch + 1) * CQ],
                            o_sb[:])
    nc.finalize()
    return nc


_NC1 = None
_NC2 = None


def _get_ncs():
    global _NC1, _NC2
    if _NC1 is None:
        _NC1 = _build_neff1()
        _NC2 = _build_neff2()
    return _NC1, _NC2


def _ensure_trace_hook():
    """Best-effort NTFF profiling shim (test harness only; grading runs
    without tracing). The agent image's antenv lacks axon_hooks, but the
    axon boot package exposes the ctypes equivalent."""
    try:
        from antenv.axon_hooks import get_axon_ntff_profile_hook
        return get_axon_ntff_profile_hook() is not None
    except ImportError:
        pass
    try:
        import types
        if "/root/.axon_site" not in sys.path:
            sys.path.insert(0, "/root/.axon_site")
        from trn_agent_boot.trn_boot import _ntff_profile_via_ctypes
        hook = _ntff_profile_via_ctypes("/opt/axon/libaxon_pjrt.so")
        if hook is None:
            return False
        import antenv
        mod = types.ModuleType("antenv.axon_hooks")
        mod.get_axon_ntff_profile_hook = lambda: hook
        mod.set_axon_ntff_profile_hook = lambda h: None
        sys.modules["antenv.axon_hooks"] = mod
        antenv.axon_hooks = mod
        from concourse import bass_utils as _bu
        _bu.upload_artifacts = lambda tmpdir: ""
        return True
    except Exception:
        return False


def _run(nc, in_maps, cores, trace, tag):
    if trace:
        try:
            r = run_bass_kernel_spmd(nc, in_maps, cores, trace=True)
            LAST_EXEC[tag] = r.exec_time_ns
            LAST_RESULTS[tag] = r
            return r
        except Exception as e:
            print(f"trace run failed ({e!r}); retrying without trace")
    return run_bass_kernel_spmd(nc, in_maps, cores)


def kernel(node_features, adjacency_matrix, mask_father, neighbor_count,
           mask_hadamard, linear_w, linear_b, weight):
    nc1, nc2 = _get_ncs()
    trace = bool(int(os.environ.get("BASS_KERNEL_TRACE", "0"))) and _ensure_trace_hook()
    cores = list(range(M))
    bf = ml_dtypes.bfloat16
    f8 = ml_dtypes.float8_e4m3

    nf = np.asarray(node_features, dtype=np.float32)
    A = np.asarray(adjacency_matrix, dtype=np.float32)
    Ao = np.asarray(mask_father, dtype=np.float32)[:, 0, :]
    S = np.asarray(mask_hadamard, dtype=np.float32)[:, 0, :]
    ncnt = np.asarray(neighbor_count, dtype=np.float32)
    lw = np.asarray(linear_w, dtype=np.float32)
    lb = np.asarray(linear_b, dtype=np.float32)
    W = np.asarray(weight, dtype=np.float32)

    # host-side weight fusion: wf = nf @ FW + fb
    FW = np.ascontiguousarray(lw.T) @ W                    # [F_RAW, C]
    fb = lb @ W                                            # [C]
    fw_b = FW.astype(bf)
    fbt = np.ascontiguousarray(np.broadcast_to(fb[None, :], (128, C)))

    # ---- launch 1: wf rows (bf16) ----
    in1 = []
    for m in range(M):
        nfT = np.ascontiguousarray(nf[m * JB:(m + 1) * JB, :].T).astype(bf)
        in1.append({"nfT": nfT, "fw": fw_b, "fbt": fbt})
    r1 = _run(nc1, in1, cores, trace, "neff1")
    wf_parts = [np.asarray(r1.results[m]["wfb"]) for m in range(M)]
    wfd = np.ascontiguousarray(np.concatenate(wf_parts, axis=0))   # [N, C] bf16

    # ---- launch 2: graph conv ----
    A8 = A.astype(f8)
    inv2 = (1.0 / np.square(ncnt.astype(np.float64)))[:, 0].astype(np.float32)
    in2 = []
    for m in range(M):
        sl = slice(m * JB, (m + 1) * JB)
        in2.append({
            "a8": A8,
            "ao8": np.ascontiguousarray(Ao[:, sl]).astype(f8),
            "stT": np.ascontiguousarray(S[:, sl].T).astype(bf),
            "wfd": wfd,
            "wfs": wf_parts[m].astype(np.float32) * inv2[None, :],
        })
    r2 = _run(nc2, in2, cores, trace, "neff2")

    out = np.empty((C, N), dtype=np.float32)
    for m in range(M):
        out[:, m * JB:(m + 1) * JB] = np.asarray(r2.results[m]["outc"]).T
    return out


# revision 8
# speedup vs baseline: 2.2737x; 1.3900x over previous
"""Distributed Bass kernel for nn_Interaction_GraphConvolution.

Math (reference):
    x  = node_features @ linear_w.T + linear_b          [N, IN_F]
    wf = x @ weight                                     [N, C]
    G  = mask_father[:,0,:].T @ adjacency               [N, N]
    P  = G * mask_hadamard[:,0,:].T                     [N, N]
    out[c, j] = wf[j,c] * (P @ wf)[j,c] / neighbor_count[c]^2

Host folds the two linear layers: FW = lw.T @ W, fb = lb @ W, so
wf = nf @ FW + fb (one on-device GEMM).  Output columns j (node dim)
are split across 8 cores, 512 each.  Two SPMD launches:
  NEFF-1: core m computes wf rows J_m (512 rows) in bf16.
  NEFF-2: core m computes G rows J_m (fp8 DoubleRow - adjacency is 0/1,
          exact), multiplies by S^T, transposes to P^T on device, then
          out[j in J_m, c] = (P@wf)[j,c] * (wf[j,c]/ncnt[c]^2).
Matmuls keep one 128x128 stationary tile across many 512-wide moving
chunks so LDWEIGHTS amortizes instead of serializing per matmul.
"""

import os
import sys

sys.path.insert(0, "/opt/trn_rl_repo")

import numpy as np
import ml_dtypes

from concourse import bass, bacc, mybir, tile
from concourse.bass_utils import run_bass_kernel_spmd
from concourse.masks import make_identity

F32 = mybir.dt.float32
BF16 = mybir.dt.bfloat16
F8E4 = mybir.dt.float8e4
DR = mybir.MatmulPerfMode.DoubleRow

N = 4096       # nodes (== out channels C)
F_RAW = 512    # raw feature dim
IN_F = 1024    # hidden dim
C = 4096       # out channels
M = 8          # cores
JB = N // M    # 512 output columns per core

LAST_EXEC = {}
LAST_RESULTS = {}


def _build_neff1():
    """Per core: wf[J_m, :] = nf[J_m] @ FW + fb, output bf16.

    Inputs: nfT [F_RAW, JB] bf16 (nf rows J_m, transposed),
    fw [F_RAW, C] bf16 (host-fused lw.T @ W), fbt [128, C] f32
    (fb broadcast to all partitions).  Output: wfb [JB, C] bf16.
    """
    nc = bacc.Bacc()
    nfT_d = nc.dram_tensor("nfT", [F_RAW, JB], BF16, kind="ExternalInput")
    fw_d = nc.dram_tensor("fw", [F_RAW, C], BF16, kind="ExternalInput")
    fbt_d = nc.dram_tensor("fbt", [128, C], F32, kind="ExternalInput")
    wf_d = nc.dram_tensor("wfb", [JB, C], BF16, kind="ExternalOutput")

    NRB = F_RAW // 128  # 4 contraction blocks
    NJB = JB // 128     # 4 j blocks
    CH = 2048           # c half

    with tile.TileContext(nc) as tc:
        with tc.tile_pool(name="const", bufs=1) as constp, \
             tc.tile_pool(name="fwp", bufs=2) as fwp, \
             tc.tile_pool(name="ps", bufs=8, space=bass.MemorySpace.PSUM) as psp, \
             tc.tile_pool(name="io", bufs=3) as iop:
            nfT_t = constp.tile([128, NRB, JB], BF16)
            for rb in range(NRB):
                nc.sync.dma_start(nfT_t[:, rb, :], nfT_d[rb * 128:(rb + 1) * 128, :])
            fbt_t = constp.tile([128, C], F32)
            nc.sync.dma_start(fbt_t[:], fbt_d[:])
            for ch in range(C // CH):
                fw_t = fwp.tile([128, NRB, CH], BF16, tag="fw", name=f"fw{ch}")
                for rb in range(NRB):
                    nc.sync.dma_start(
                        fw_t[:, rb, :],
                        fw_d[rb * 128:(rb + 1) * 128, ch * CH:(ch + 1) * CH])
                for jb in range(NJB):
                    ps = [psp.tile([128, 512], F32, tag="ps", name=f"ps{ch}_{jb}_{i}")
                          for i in range(4)]
                    for rb in range(NRB):
                        lhs = nfT_t[:, rb, jb * 128:(jb + 1) * 128]
                        for cc in range(4):
                            nc.tensor.matmul(
                                ps[cc][:], lhs,
                                fw_t[:, rb, cc * 512:(cc + 1) * 512],
                                start=(rb == 0), stop=(rb == NRB - 1))
                    o_sb = iop.tile([128, CH], BF16, tag="o_sb", name=f"o{ch}_{jb}")
                    for cc in range(4):
                        nc.vector.tensor_add(
                            o_sb[:, cc * 512:(cc + 1) * 512], ps[cc][:],
                            fbt_t[:, ch * CH + cc * 512: ch * CH + (cc + 1) * 512])
                    nc.sync.dma_start(
                        wf_d[jb * 128:(jb + 1) * 128, ch * CH:(ch + 1) * CH],
                        o_sb[:])
    nc.finalize()
    return nc


def _build_neff2():
    """Per core: out[j in J_m, c] = (P @ wf)[j, c] * wfs[j, c].

    Uses wf = nf @ FW + fb to factor the big GEMM:
        P @ wf = (P @ nf) @ FW + rowsum(P) x fb
    which is 2*N^2*F_RAW + 2*N*F_RAW*C flops instead of 2*N^2*C (4x less).

    Inputs: a8 [N, N] fp8e4 (adjacency), ao8 [N, JB] fp8e4 (mask_father
    cols J_m), stT [JB, N] bf16 (mask_hadamard cols J_m, transposed),
    nfb [N, F_RAW] bf16 (node features), fwt [F_RAW, C] bf16 (lw.T @ W),
    fbt [128, C] f32 (lb @ W broadcast), wfs [JB, C] f32
    (wf rows J_m * 1/ncnt^2).
    Output: outc [JB, C] f32 (transposed vs final; host untransposes).
    """
    nc = bacc.Bacc()
    a_d = nc.dram_tensor("a8", [N, N], F8E4, kind="ExternalInput")
    ao_d = nc.dram_tensor("ao8", [N, JB], F8E4, kind="ExternalInput")
    st_d = nc.dram_tensor("stT", [JB, N], BF16, kind="ExternalInput")
    nf_d = nc.dram_tensor("nfb", [N, F_RAW], BF16, kind="ExternalInput")
    fw_d = nc.dram_tensor("fwt", [F_RAW, C], BF16, kind="ExternalInput")
    fbt_d = nc.dram_tensor("fbt", [128, C], F32, kind="ExternalInput")
    wfs_d = nc.dram_tensor("wfs", [JB, C], F32, kind="ExternalInput")
    out_d = nc.dram_tensor("outc", [JB, C], F32, kind="ExternalOutput")

    NKP = N // 256    # 16 k-pairs (DoubleRow contracts 256 per pass)
    NIB = N // 128    # 32 i-blocks
    NJB = JB // 128   # 4 j-blocks
    NIC = N // 512    # 8 i-chunks of 512
    NRB = F_RAW // 128  # 4 r-blocks
    CH = 2048         # c half

    with tile.TileContext(nc) as tc:
        with tc.tile_pool(name="const", bufs=1) as constp:
            ident = constp.tile([128, 128], BF16)
            make_identity(nc, ident[:])
            # P^T, [i, j] with i on partitions: ptT[:, ib, j] = P[j, ib*128+p]
            ptT_t = constp.tile([128, NIB, JB], BF16)
            rsum_t = constp.tile([128, NJB], F32)  # rowsum(P), col per j-block

            # ---- phase G: G rows J_m (fp8 DoubleRow), *S^T, transpose ----
            with tc.tile_pool(name="ga", bufs=1) as gap, \
                 tc.tile_pool(name="stp", bufs=1) as stp, \
                 tc.tile_pool(name="pgp", bufs=1) as pgp:
                aot_t = gap.tile([128, NKP, 2, JB], F8E4)
                for kb in range(2 * NKP):
                    nc.sync.dma_start(
                        aot_t[:, kb // 2, kb % 2, :],
                        ao_d[kb * 128:(kb + 1) * 128, :])
                a_t = gap.tile([128, NKP, 2, N], F8E4)
                for kb in range(2 * NKP):
                    nc.sync.dma_start(
                        a_t[:, kb // 2, kb % 2, :],
                        a_d[kb * 128:(kb + 1) * 128, :])

                for jb in range(NJB):
                    st_t = stp.tile([128, N], BF16, tag="st", name=f"st{jb}")
                    nc.sync.dma_start(st_t[:], st_d[jb * 128:(jb + 1) * 128, :])
                    pg_sb = pgp.tile([128, N], BF16, tag="pg", name=f"pg{jb}")
                    with tc.tile_pool(name=f"psG{jb}", bufs=8,
                                      space=bass.MemorySpace.PSUM) as psgp:
                        psg = [psgp.tile([128, 512], F32, tag="g",
                                         name=f"g{jb}_{i}") for i in range(NIC)]
                        for kp in range(NKP):
                            lhs = aot_t[:, kp, :, jb * 128:(jb + 1) * 128]
                            for ic in range(NIC):
                                nc.tensor.matmul(
                                    psg[ic][:], lhs,
                                    a_t[:, kp, :, ic * 512:(ic + 1) * 512],
                                    start=(kp == 0), stop=(kp == NKP - 1),
                                    perf_mode=DR)
                        for ic in range(NIC):
                            nc.vector.tensor_mul(
                                pg_sb[:, ic * 512:(ic + 1) * 512], psg[ic][:],
                                st_t[:, ic * 512:(ic + 1) * 512])
                    nc.vector.reduce_sum(
                        rsum_t[:, jb:jb + 1], pg_sb[:],
                        axis=mybir.AxisListType.X)
                    with tc.tile_pool(name=f"psT{jb}", bufs=4,
                                      space=bass.MemorySpace.PSUM) as pstp:
                        for ib in range(NIB):
                            pst = pstp.tile([128, 128], BF16, tag="t",
                                            name=f"t{jb}_{ib}")
                            nc.tensor.transpose(
                                pst[:], pg_sb[:, ib * 128:(ib + 1) * 128],
                                ident[:])
                            nc.scalar.copy(
                                ptT_t[:, ib, jb * 128:(jb + 1) * 128], pst[:])

            # ---- phase X: XT[r, j] = sum_i nf[i, r] * P^T[i, j] ----
            with tc.tile_pool(name="nfp", bufs=1) as nfp, \
                 tc.tile_pool(name="fwp", bufs=1) as fwp, \
                 tc.tile_pool(name="xtp", bufs=1) as xtp:
                nf_t = nfp.tile([128, NIB, F_RAW], BF16)
                for ib in range(NIB):
                    nc.sync.dma_start(
                        nf_t[:, ib, :], nf_d[ib * 128:(ib + 1) * 128, :])
                fw_t = fwp.tile([128, NRB, C], BF16)
                for rb in range(NRB):
                    nc.sync.dma_start(
                        fw_t[:, rb, :], fw_d[rb * 128:(rb + 1) * 128, :])
                fbt_t = fwp.tile([128, C], F32)
                nc.sync.dma_start(fbt_t[:], fbt_d[:])
                xt_sb = xtp.tile([128, NRB, JB], BF16)
                with tc.tile_pool(name="psX", bufs=4,
                                  space=bass.MemorySpace.PSUM) as psxp:
                    psx = [psxp.tile([128, JB], F32, tag="x", name=f"x{i}")
                           for i in range(NRB)]
                    for ib in range(NIB):
                        for rb in range(NRB):
                            nc.tensor.matmul(
                                psx[rb][:],
                                nf_t[:, ib, rb * 128:(rb + 1) * 128],
                                ptT_t[:, ib, :],
                                start=(ib == 0), stop=(ib == NIB - 1))
                    for rb in range(NRB):
                        nc.vector.tensor_copy(xt_sb[:, rb, :], psx[rb][:])

                # ---- phase O: out[j, c] = (XT.T @ FW + rsum*fb) * wfs ----
                with tc.tile_pool(name="wfsp", bufs=2) as wfsp, \
                     tc.tile_pool(name="epp", bufs=2) as epp, \
                     tc.tile_pool(name="psO", bufs=8,
                                  space=bass.MemorySpace.PSUM) as psop:
                    for jb in range(NJB):
                        for ch in range(C // CH):
                            ps2 = [psop.tile([128, 512], F32, tag="po",
                                             name=f"po{jb}_{ch}_{i}")
                                   for i in range(4)]
                            for rb in range(NRB):
                                lhs = xt_sb[:, rb, jb * 128:(jb + 1) * 128]
                                for cc in range(4):
                                    nc.tensor.matmul(
                                        ps2[cc][:], lhs,
                                        fw_t[:, rb, ch * CH + cc * 512:
                                             ch * CH + (cc + 1) * 512],
                                        start=(rb == 0), stop=(rb == NRB - 1))
                            wfs_t = wfsp.tile([128, CH], F32, tag="wfs",
                                              name=f"wfs{jb}_{ch}")
                            nc.sync.dma_start(
                                wfs_t[:],
                                wfs_d[jb * 128:(jb + 1) * 128,
                                      ch * CH:(ch + 1) * CH])
                            acc2 = epp.tile([128, CH], F32, tag="a2",
                                            name=f"a2{jb}_{ch}")
                            nc.scalar.activation(
                                acc2[:], fbt_t[:, ch * CH:(ch + 1) * CH],
                                mybir.ActivationFunctionType.Identity,
                                bias=0.0, scale=rsum_t[:, jb:jb + 1])
                            t_sb = epp.tile([128, CH], F32, tag="t",
                                            name=f"t{jb}_{ch}")
                            for cc in range(4):
                                nc.vector.tensor_add(
                                    t_sb[:, cc * 512:(cc + 1) * 512],
                                    ps2[cc][:],
                                    acc2[:, cc * 512:(cc + 1) * 512])
                            o_sb = epp.tile([128, CH], F32, tag="o",
                                            name=f"o{jb}_{ch}")
                            nc.vector.tensor_mul(o_sb[:], t_sb[:], wfs_t[:])
                            nc.sync.dma_start(
                                out_d[jb * 128:(jb + 1) * 128,
                                      ch * CH:(ch + 1) * CH],
                                o_sb[:])
    nc.finalize()
    return nc


_NC1 = None
_NC2 = None


def _get_ncs():
    global _NC1, _NC2
    if _NC1 is None:
        _NC1 = _build_neff1()
        _NC2 = _build_neff2()
    return _NC1, _NC2


def _ensure_trace_hook():
    """Best-effort NTFF profiling shim (test harness only; grading runs
    without tracing). The agent image's antenv lacks axon_hooks, but the
    axon boot package exposes the ctypes equivalent."""
    try:
        from antenv.axon_hooks import get_axon_ntff_profile_hook
        return get_axon_ntff_profile_hook() is not None
    except ImportError:
        pass
    try:
        import types
        if "/root/.axon_site" not in sys.path:
            sys.path.insert(0, "/root/.axon_site")
        from trn_agent_boot.trn_boot import _ntff_profile_via_ctypes
        hook = _ntff_profile_via_ctypes("/opt/axon/libaxon_pjrt.so")
        if hook is None:
            return False
        import antenv
        mod = types.ModuleType("antenv.axon_hooks")
        mod.get_axon_ntff_profile_hook = lambda: hook
        mod.set_axon_ntff_profile_hook = lambda h: None
        sys.modules["antenv.axon_hooks"] = mod
        antenv.axon_hooks = mod
        from concourse import bass_utils as _bu
        _bu.upload_artifacts = lambda tmpdir: ""
        return True
    except Exception:
        return False


def _run(nc, in_maps, cores, trace, tag):
    if trace:
        try:
            r = run_bass_kernel_spmd(nc, in_maps, cores, trace=True)
            LAST_EXEC[tag] = r.exec_time_ns
            LAST_RESULTS[tag] = r
            return r
        except Exception as e:
            print(f"trace run failed ({e!r}); retrying without trace")
    return run_bass_kernel_spmd(nc, in_maps, cores)


def kernel(node_features, adjacency_matrix, mask_father, neighbor_count,
           mask_hadamard, linear_w, linear_b, weight):
    nc1, nc2 = _get_ncs()
    trace = bool(int(os.environ.get("BASS_KERNEL_TRACE", "0"))) and _ensure_trace_hook()
    cores = list(range(M))
    bf = ml_dtypes.bfloat16
    f8 = ml_dtypes.float8_e4m3

    nf = np.asarray(node_features, dtype=np.float32)
    A = np.asarray(adjacency_matrix, dtype=np.float32)
    Ao = np.asarray(mask_father, dtype=np.float32)[:, 0, :]
    S = np.asarray(mask_hadamard, dtype=np.float32)[:, 0, :]
    ncnt = np.asarray(neighbor_count, dtype=np.float32)
    lw = np.asarray(linear_w, dtype=np.float32)
    lb = np.asarray(linear_b, dtype=np.float32)
    W = np.asarray(weight, dtype=np.float32)

    # host-side weight fusion: wf = nf @ FW + fb
    FW = np.ascontiguousarray(lw.T) @ W                    # [F_RAW, C]
    fb = lb @ W                                            # [C]
    fw_b = FW.astype(bf)
    fbt = np.ascontiguousarray(np.broadcast_to(fb[None, :], (128, C)))

    # ---- launch 1: wf rows (bf16) ----
    in1 = []
    for m in range(M):
        nfT = np.ascontiguousarray(nf[m * JB:(m + 1) * JB, :].T).astype(bf)
        in1.append({"nfT": nfT, "fw": fw_b, "fbt": fbt})
    r1 = _run(nc1, in1, cores, trace, "neff1")
    wf_parts = [np.asarray(r1.results[m]["wfb"]) for m in range(M)]

    # ---- launch 2: graph conv ----
    A8 = A.astype(f8)
    nfb = nf.astype(bf)
    inv2 = (1.0 / np.square(ncnt.astype(np.float64)))[:, 0].astype(np.float32)
    in2 = []
    for m in range(M):
        sl = slice(m * JB, (m + 1) * JB)
        in2.append({
            "a8": A8,
            "ao8": np.ascontiguousarray(Ao[:, sl]).astype(f8),
            "stT": np.ascontiguousarray(S[:, sl].T).astype(bf),
            "nfb": nfb,
            "fwt": fw_b,
            "fbt": fbt,
            "wfs": wf_parts[m].astype(np.float32) * inv2[None, :],
        })
    r2 = _run(nc2, in2, cores, trace, "neff2")

    out = np.empty((C, N), dtype=np.float32)
    for m in range(M):
        out[:, m * JB:(m + 1) * JB] = np.asarray(r2.results[m]["outc"]).T
    return out


# revision 10
# speedup vs baseline: 3.1916x; 1.4037x over previous
"""Distributed Bass kernel for nn_Interaction_GraphConvolution.

Math (reference):
    x  = node_features @ linear_w.T + linear_b          [N, IN_F]
    wf = x @ weight                                     [N, C]
    G  = mask_father[:,0,:].T @ adjacency               [N, N]
    P  = G * mask_hadamard[:,0,:].T                     [N, N]
    out[c, j] = wf[j,c] * (P @ wf)[j,c] / neighbor_count[c]^2

Host folds the two linear layers (FW = lw.T @ W, fb = lb @ W) and the
normalization (FW2 = FW / ncnt^2, fb2 = fb / ncnt^2).  The big GEMM is
factored through the linear layer:
    P @ wf = (P @ nf) @ FW + rowsum(P) x fb
which is 2*N^2*F_RAW + 2*N*F_RAW*C flops instead of 2*N^2*C (4x less).

Single SPMD launch, output columns j (node dim) split across 8 cores:
  phase G: G rows J_m via fp8e4 DoubleRow (adjacency/mask are 0/1 -
           exact), multiply by S^T, DMA-XBAR transpose into P^T.
  phase X: XT = nf^T @ P^T (bf16).
  phase O: out[j,c] = (XT.T@FW + rsum*fb) * (nfT.T@FW2 + fb2).
Matmuls keep one 128x128 stationary across multiple 512-wide moving
chunks so LDWEIGHTS amortizes.  Inputs are host-packed into SBUF layout
so every DMA moves >=4KB per partition line (descriptor-rate bound).
"""

import os
import sys

sys.path.insert(0, "/opt/trn_rl_repo")

import numpy as np
import ml_dtypes

from concourse import bass, bacc, mybir, tile
from concourse.bass_utils import run_bass_kernel_spmd

F32 = mybir.dt.float32
BF16 = mybir.dt.bfloat16
F8E4 = mybir.dt.float8e4
DR = mybir.MatmulPerfMode.DoubleRow

N = 4096       # nodes (== out channels C)
F_RAW = 512    # raw feature dim
IN_F = 1024    # hidden dim
C = 4096       # out channels
M = 8          # cores
JB = N // M    # 512 output columns per core

NKP = N // 256    # 16 k-pairs (DoubleRow contracts 256 per pass)
NKB = N // 128    # 32 k-blocks
NIB = N // 128    # 32 i-blocks
NJB = JB // 128   # 4 j-blocks
NIC = N // 512    # 8 i-chunks of 512
NRB = F_RAW // 128  # 4 r-blocks
CQ = 1024         # c quarter
NCQ = C // CQ     # 4

LAST_EXEC = {}
LAST_RESULTS = {}


def _build_neff():
    nc = bacc.Bacc()
    a_d = nc.dram_tensor("a8", [128, NKB, N], F8E4, kind="ExternalInput")
    ao_d = nc.dram_tensor("ao8", [128, NKB, JB], F8E4, kind="ExternalInput")
    st_d = nc.dram_tensor("stT", [JB, N], BF16, kind="ExternalInput")
    nf_d = nc.dram_tensor("nfb", [128, NIB, F_RAW], BF16, kind="ExternalInput")
    nfT_d = nc.dram_tensor("nfT", [128, NRB, JB], BF16, kind="ExternalInput")
    fw_d = nc.dram_tensor("fwt", [128, NRB, C], BF16, kind="ExternalInput")
    fw2_d = nc.dram_tensor("fw2", [128, NRB, C], BF16, kind="ExternalInput")
    fbt_d = nc.dram_tensor("fbt", [128, C], F32, kind="ExternalInput")
    fb2_d = nc.dram_tensor("fb2", [128, C], F32, kind="ExternalInput")
    out_d = nc.dram_tensor("outc", [JB, C], F32, kind="ExternalOutput")

    with tile.TileContext(nc) as tc:
        with tc.tile_pool(name="const", bufs=1) as constp:
            # P^T: ptT[p, ib, j] = P[j, ib*128+p], i on partitions
            ptT_t = constp.tile([128, NIB, JB], BF16)
            rsum_t = constp.tile([128, NJB], F32)  # rowsum(P), col per j-block

            # ---- phase G: G rows J_m (fp8 DoubleRow), *S^T, transpose ----
            with tc.tile_pool(name="ga", bufs=1) as gap, \
                 tc.tile_pool(name="stp", bufs=1) as stp, \
                 tc.tile_pool(name="pgp", bufs=2) as pgp, \
                 tc.tile_pool(name="psG", bufs=8,
                              space=bass.MemorySpace.PSUM) as psgp:
                aot_t = gap.tile([128, NKP, 2, JB], F8E4)
                nc.sync.dma_start(aot_t[:], ao_d[:])
                a_t = gap.tile([128, NKP, 2, N], F8E4)
                for kp in range(NKP):
                    nc.sync.dma_start(
                        a_t[:, kp, :, :], a_d[:, 2 * kp:2 * kp + 2, :])

                for jb in range(NJB):
                    st_t = stp.tile([128, N], BF16, tag="st", name=f"st{jb}")
                    nc.sync.dma_start(st_t[:], st_d[jb * 128:(jb + 1) * 128, :])
                    pg_sb = pgp.tile([128, N], BF16, tag="pg", name=f"pg{jb}")
                    psg = [psgp.tile([128, 512], F32, tag="g",
                                     name=f"g{jb}_{i}") for i in range(NIC)]
                    for kp in range(NKP):
                        lhs = aot_t[:, kp, :, jb * 128:(jb + 1) * 128]
                        for ic in range(NIC):
                            nc.tensor.matmul(
                                psg[ic][:], lhs,
                                a_t[:, kp, :, ic * 512:(ic + 1) * 512],
                                start=(kp == 0), stop=(kp == NKP - 1),
                                perf_mode=DR)
                    for ic in range(NIC):
                        nc.vector.tensor_mul(
                            pg_sb[:, ic * 512:(ic + 1) * 512], psg[ic][:],
                            st_t[:, ic * 512:(ic + 1) * 512])
                    nc.vector.reduce_sum(
                        rsum_t[:, jb:jb + 1], pg_sb[:],
                        axis=mybir.AxisListType.X)
                    nc.sync.dma_start_transpose(
                        ptT_t[:, :, jb * 128:(jb + 1) * 128], pg_sb[:])

            # ---- phase X: XT[r, j] = sum_i nf[i, r] * P^T[i, j] ----
            with tc.tile_pool(name="nfp", bufs=1) as nfp, \
                 tc.tile_pool(name="fwp", bufs=1) as fwp, \
                 tc.tile_pool(name="xtp", bufs=1) as xtp:
                nf_t = nfp.tile([128, NIB, F_RAW], BF16)
                nc.sync.dma_start(nf_t[:], nf_d[:])
                nfT_t = nfp.tile([128, NRB, JB], BF16)
                nc.sync.dma_start(nfT_t[:], nfT_d[:])
                fw_t = fwp.tile([128, NRB, C], BF16)
                nc.sync.dma_start(fw_t[:], fw_d[:])
                fw2_t = fwp.tile([128, NRB, C], BF16)
                nc.sync.dma_start(fw2_t[:], fw2_d[:])
                fbt_t = fwp.tile([128, C], F32)
                nc.sync.dma_start(fbt_t[:], fbt_d[:])
                fb2_t = fwp.tile([128, C], F32)
                nc.sync.dma_start(fb2_t[:], fb2_d[:])
                xt_sb = xtp.tile([128, NRB, JB], BF16)
                with tc.tile_pool(name="psX", bufs=4,
                                  space=bass.MemorySpace.PSUM) as psxp:
                    psx = [psxp.tile([128, JB], F32, tag="x", name=f"x{i}")
                           for i in range(NRB)]
                    for ib in range(NIB):
                        for rb in range(NRB):
                            nc.tensor.matmul(
                                psx[rb][:],
                                nf_t[:, ib, rb * 128:(rb + 1) * 128],
                                ptT_t[:, ib, :],
                                start=(ib == 0), stop=(ib == NIB - 1))
                    for rb in range(NRB):
                        nc.vector.tensor_copy(xt_sb[:, rb, :], psx[rb][:])

                # ---- phase O: out = (XT.T@FW + rsum*fb) * (nfT.T@FW2 + fb2)
                with tc.tile_pool(name="epp", bufs=2) as epp, \
                     tc.tile_pool(name="psA", bufs=4,
                                  space=bass.MemorySpace.PSUM) as psap, \
                     tc.tile_pool(name="psW", bufs=4,
                                  space=bass.MemorySpace.PSUM) as pswp:
                    for jb in range(NJB):
                        for cq in range(NCQ):
                            psa = [psap.tile([128, 512], F32, tag="pa",
                                             name=f"pa{jb}_{cq}_{i}")
                                   for i in range(2)]
                            psw = [pswp.tile([128, 512], F32, tag="pw",
                                             name=f"pw{jb}_{cq}_{i}")
                                   for i in range(2)]
                            for rb in range(NRB):
                                lhsa = xt_sb[:, rb, jb * 128:(jb + 1) * 128]
                                for cc in range(2):
                                    nc.tensor.matmul(
                                        psa[cc][:], lhsa,
                                        fw_t[:, rb, cq * CQ + cc * 512:
                                             cq * CQ + (cc + 1) * 512],
                                        start=(rb == 0), stop=(rb == NRB - 1))
                                lhsw = nfT_t[:, rb, jb * 128:(jb + 1) * 128]
                                for cc in range(2):
                                    nc.tensor.matmul(
                                        psw[cc][:], lhsw,
                                        fw2_t[:, rb, cq * CQ + cc * 512:
                                              cq * CQ + (cc + 1) * 512],
                                        start=(rb == 0), stop=(rb == NRB - 1))
                            sl = slice(cq * CQ, (cq + 1) * CQ)
                            acc2 = epp.tile([128, CQ], F32, tag="a2",
                                            name=f"a2{jb}_{cq}")
                            nc.scalar.activation(
                                acc2[:], fbt_t[:, sl],
                                mybir.ActivationFunctionType.Identity,
                                bias=0.0, scale=rsum_t[:, jb:jb + 1])
                            wfsc = epp.tile([128, CQ], F32, tag="wsc",
                                            name=f"wsc{jb}_{cq}")
                            t_sb = epp.tile([128, CQ], F32, tag="t",
                                            name=f"t{jb}_{cq}")
                            for cc in range(2):
                                ccs = slice(cc * 512, (cc + 1) * 512)
                                nc.vector.tensor_add(
                                    wfsc[:, ccs], psw[cc][:],
                                    fb2_t[:, cq * CQ + cc * 512:
                                          cq * CQ + (cc + 1) * 512])
                                nc.vector.tensor_add(
                                    t_sb[:, ccs], psa[cc][:], acc2[:, ccs])
                            o_sb = epp.tile([128, CQ], F32, tag="o",
                                            name=f"o{jb}_{cq}")
                            nc.vector.tensor_mul(o_sb[:], t_sb[:], wfsc[:])
                            nc.sync.dma_start(
                                out_d[jb * 128:(jb + 1) * 128, sl], o_sb[:])
    nc.finalize()
    return nc


_NC = None


def _get_nc():
    global _NC
    if _NC is None:
        _NC = _build_neff()
    return _NC


def _ensure_trace_hook():
    """Best-effort NTFF profiling shim (test harness only; grading runs
    without tracing)."""
    try:
        from antenv.axon_hooks import get_axon_ntff_profile_hook
        return get_axon_ntff_profile_hook() is not None
    except ImportError:
        pass
    try:
        import types
        if "/root/.axon_site" not in sys.path:
            sys.path.insert(0, "/root/.axon_site")
        from trn_agent_boot.trn_boot import _ntff_profile_via_ctypes
        hook = _ntff_profile_via_ctypes("/opt/axon/libaxon_pjrt.so")
        if hook is None:
            return False
        import antenv
        mod = types.ModuleType("antenv.axon_hooks")
        mod.get_axon_ntff_profile_hook = lambda: hook
        mod.set_axon_ntff_profile_hook = lambda h: None
        sys.modules["antenv.axon_hooks"] = mod
        antenv.axon_hooks = mod
        from concourse import bass_utils as _bu
        _bu.upload_artifacts = lambda tmpdir: ""
        return True
    except Exception:
        return False


def _run(nc, in_maps, cores, trace, tag):
    if trace:
        try:
            r = run_bass_kernel_spmd(nc, in_maps, cores, trace=True)
            LAST_EXEC[tag] = r.exec_time_ns
            LAST_RESULTS[tag] = r
            return r
        except Exception as e:
            print(f"trace run failed ({e!r}); retrying without trace")
    return run_bass_kernel_spmd(nc, in_maps, cores)


def _pack(x, nblk):
    """[nblk*128, F] -> [128, nblk, F] (SBUF layout, row-block p-major)."""
    f = x.shape[1]
    return np.ascontiguousarray(
        x.reshape(nblk, 128, f).transpose(1, 0, 2))


def kernel(node_features, adjacency_matrix, mask_father, neighbor_count,
           mask_hadamard, linear_w, linear_b, weight):
    nc = _get_nc()
    trace = bool(int(os.environ.get("BASS_KERNEL_TRACE", "0"))) and _ensure_trace_hook()
    cores = list(range(M))
    bf = ml_dtypes.bfloat16
    f8 = ml_dtypes.float8_e4m3

    nf = np.asarray(node_features, dtype=np.float32)
    A = np.asarray(adjacency_matrix, dtype=np.float32)
    Ao = np.asarray(mask_father, dtype=np.float32)[:, 0, :]
    S = np.asarray(mask_hadamard, dtype=np.float32)
    ncnt = np.asarray(neighbor_count, dtype=np.float32)
    lw = np.asarray(linear_w, dtype=np.float32)
    lb = np.asarray(linear_b, dtype=np.float32)
    W = np.asarray(weight, dtype=np.float32)

    FW = np.ascontiguousarray(lw.T) @ W                    # [F_RAW, C]
    fb = lb @ W                                            # [C]
    inv2 = (1.0 / np.square(ncnt.astype(np.float64)))[:, 0].astype(np.float32)
    FW2 = FW * inv2[None, :]
    fb2 = fb * inv2

    a_re = _pack(A.astype(f8), NKB)                        # [128, 32, N]
    nf_re = _pack(nf.astype(bf), NIB)                      # [128, 32, F_RAW]
    fw_re = _pack(FW.astype(bf), NRB)                      # [128, 4, C]
    fw2_re = _pack(FW2.astype(bf), NRB)
    fbt = np.ascontiguousarray(np.broadcast_to(fb[None, :], (128, C)))
    fb2t = np.ascontiguousarray(np.broadcast_to(fb2[None, :], (128, C)))

    in_maps = []
    for m in range(M):
        sl = slice(m * JB, (m + 1) * JB)
        in_maps.append({
            "a8": a_re,
            "ao8": _pack(np.ascontiguousarray(Ao[:, sl]).astype(f8), NKB),
            "stT": np.ascontiguousarray(S[:, 0, sl].T).astype(bf),
            "nfb": nf_re,
            "nfT": _pack(np.ascontiguousarray(nf[sl].T).astype(bf), NRB),
            "fwt": fw_re,
            "fw2": fw2_re,
            "fbt": fbt,
            "fb2": fb2t,
        })
    r = _run(nc, in_maps, cores, trace, "neff")

    out = np.empty((C, N), dtype=np.float32)
    for m in range(M):
        out[:, m * JB:(m + 1) * JB] = np.asarray(r.results[m]["outc"]).T
    return out
